# revision 11
# baseline (speedup 1.0000x reference)
"""Trainium2 Bass kernel for nn_EquivarianceNetwork (grouped 4-layer MLP).

Math (per sample b, TWO_N=16 groups, D=64):
  xr = x.reshape(B, 16, 64)
  scalars[b, n, m] = <xr[b,n], xr[b,m]>                    # 136 unique pairs
  per group l: h = tanh(...W0/W1/W2...), coeffs = h @ W3 + b3   # [B, 16]
  out[b, l*64:(l+1)*64] = sum_n coeffs[l,b,n] * xr[b,n]

Distribution: data-parallel over batch across 8 cores (weights replicated).
Per core B_local = 2048.

Design notes (v1):
  - All matmuls fp16 (1 cyc/row on PE, same rate as fp32r, 11-bit mantissa);
    weights host-pretiled to [K-part, kt, M] fp16 -> half the DMA bytes.
  - x is host-transposed to d-major per sample (xdm[b, d*16+n]), fp16. One
    resident SBUF copy serves both the Gram products and the final
    contraction; no per-stage x re-DMA.
  - Scalars: only the 136 unique pairs, ordered by diagonal offset dl so
    each Gram reduce writes a contiguous slice. W0 is host-folded and
    row-reordered to match. Gram = 16 fp16 muls (2x DVE mode, d-major
    slices) + a flat fp16 halving add-tree over d (2x mode).
  - Finals: coeffs stored batch-major fp16 [128, 256]; unit (l,s) is one
    broadcast-outer mul (2x mode, ~0.7us) + one reduce-X into f32.
  - DVE/GPSIMD/ACT all end far below PE (~2.1ms); PE stream is the
    critical path.
"""
import numpy as np

import concourse.bass as bass
import concourse.mybir as mybir
import concourse.tile as tile
from concourse import bacc
from concourse.bass_utils import run_bass_kernel_spmd
from concourse.masks import make_identity

F32 = mybir.dt.float32
F16 = mybir.dt.float16
TANH = mybir.ActivationFunctionType.Tanh

N_CORES = 8
B = 16384
TWO_N = 16
D = 64
B_LOC = B // N_CORES          # 2048
N_SUB = B_LOC // 128          # 16 subtiles of 128 samples
N_BT = B_LOC // 512           # 4 batch tiles of 512 (matmul free dim)
H = 1024                      # hidden width
NPAIR = 136                   # unique Gram pairs
# pair order: j = OFF[dl] + n  <->  scal_j = <xr[n], xr[n+dl]>
OFF = [0]
for _dl in range(1, 16):
    OFF.append(OFF[-1] + (16 - _dl + 1))


def _ap3(t, off, s1, n1, s2, n2):
    return bass.AP(tensor=t.tensor, offset=t.offset + off,
                   ap=[t.ap[0], [s1, n1], [s2, n2]])


def _build_program():
    nc = bacc.Bacc()

    xdm = nc.declare_dram_parameter("xdm", [B_LOC, TWO_N * D], F16, isOutput=False)
    W0f = nc.declare_dram_parameter("W0f", [TWO_N, NPAIR, H], F16, isOutput=False)
    W1t = nc.declare_dram_parameter("W1t", [TWO_N, 128, 8 * H], F16, isOutput=False)
    W2t = nc.declare_dram_parameter("W2t", [TWO_N, 128, 8 * H], F16, isOutput=False)
    W3t = nc.declare_dram_parameter("W3t", [TWO_N, 128, 8 * TWO_N], F16,
                                    isOutput=False)
    b012 = nc.declare_dram_parameter("b012", [128, 3 * 8 * TWO_N], F32,
                                     isOutput=False)
    b3T = nc.declare_dram_parameter("b3T", [TWO_N, TWO_N], F32, isOutput=False)
    y = nc.declare_dram_parameter("y", [B_LOC, TWO_N * D], F32, isOutput=True)

    with tile.TileContext(nc) as tc:
        with tc.tile_pool(name="res", bufs=1) as res, \
             tc.tile_pool(name="gw", bufs=1) as gw, \
             tc.tile_pool(name="wk", bufs=2) as wk, \
             tc.tile_pool(name="w0p", bufs=2) as w0p, \
             tc.tile_pool(name="w12p", bufs=4) as w12p, \
             tc.tile_pool(name="w3p", bufs=2) as w3p, \
             tc.tile_pool(name="hp", bufs=2) as hp, \
             tc.tile_pool(name="fin", bufs=4) as finp, \
             tc.tile_pool(name="ps", bufs=4, space="PSUM") as ps:

            ident = res.tile([128, 128], F16)
            make_identity(nc, ident)

            # ---- x: resident d-major fp16, one tile per 128-sample subtile
            xh = [res.tile([128, TWO_N * D], F16, name=f"xh{s}")
                  for s in range(N_SUB)]
            for s in range(N_SUB):
                nc.sync.dma_start(out=xh[s], in_=xdm[128 * s:128 * (s + 1), :])

            # ---- biases: host-pretransposed; b012_all[p, li, ot, l] ----
            b012_all = res.tile([128, 3, 8, TWO_N], F32)
            nc.sync.dma_start(
                out=b012_all,
                in_=b012[:, :].rearrange("p (li ot l) -> p li ot l",
                                         li=3, ot=8))
            b3_all = res.tile([16, TWO_N], F32)   # [n, l]
            nc.sync.dma_start(out=b3_all, in_=b3T[:, :])

            # resident: transposed scalars, fp16
            scalT0 = res.tile([128, B_LOC], F16, name="scalT0")
            scalT1 = res.tile([8, B_LOC], F16, name="scalT1")
            # resident: coeffs batch-major per subtile [128, 256] (col 16l+n)
            coeff = [res.tile([128, 256], F16, name=f"coeff{s}")
                     for s in range(N_SUB)]

            # ---------------- Gram for one subtile of 128 samples ----------
            # prod[p, d, j] = xh[p, d*16+n] * xh[p, d*16+n+dl]  (j = OFF[dl]+n)
            # then halving add-tree over d (flat fp16 2x adds) -> sbm[p, 136]
            def gram(s):
                x = xh[s]
                prod = gw.tile([128, 64 * NPAIR], F16, name="prod", tag="prod")
                for dl in range(TWO_N):
                    npair = TWO_N - dl
                    meng = nc.gpsimd if dl < 3 else nc.vector
                    meng.tensor_mul(
                        _ap3(prod, OFF[dl], NPAIR, 64, 1, npair),
                        _ap3(x, 0, 16, 64, 1, npair),
                        _ap3(x, dl, 16, 64, 1, npair),
                    )
                sbm = gw.tile([128, NPAIR], F16, name="sbm", tag="sbm", bufs=2)
                src = prod
                sz = 32 * NPAIR
                for st in range(6):
                    dst = (gw.tile([128, sz], F16, name=f"t{st}", tag=f"t{st}")
                           if st < 5 else sbm)
                    nc.vector.tensor_add(dst[:, :], src[:, 0:sz],
                                         src[:, sz:2 * sz])
                    src = dst
                    sz //= 2
                pt = ps.tile([128, 128], F16, name="tpg", tag="tp", bufs=2)
                nc.tensor.transpose(pt[:, :], sbm[:, 0:128], ident)
                nc.scalar.copy(scalT0[:, 128 * s:128 * (s + 1)], pt[:, :])
                pt2 = ps.tile([8, 128], F16, name="tpg2", tag="tp2", bufs=1)
                nc.tensor.transpose(pt2[:, :], sbm[:, 128:NPAIR], ident)
                nc.scalar.copy(scalT1[:, 128 * s:128 * (s + 1)], pt2[:, :])

            # ---- final contraction for one (l, subtile):
            # y[bsub, l*64+d] = sum_n coeff[b, 16l+n] * xh[b, d*16+n]
            def final_unit(l, s, eng):
                c = coeff[s]
                prodf = finp.tile([128, TWO_N * D], F16, name="prodf",
                                  tag="prodf", bufs=3)
                eng.tensor_mul(
                    _ap3(prodf, 0, 16, 64, 1, 16),
                    _ap3(xh[s], 0, 16, 64, 1, 16),
                    _ap3(c, 16 * l, 0, 64, 1, 16),
                )
                fcol = finp.tile([128, D], F32, name="fcol", tag="fcol")
                nc.vector.tensor_reduce(
                    fcol[:, :], _ap3(prodf, 0, 16, 64, 1, 16),
                    axis=mybir.AxisListType.X, op=mybir.AluOpType.add)
                nc.sync.dma_start(
                    out=y[128 * s:128 * (s + 1), D * l:D * (l + 1)],
                    in_=fcol[:, :])

            # ---------------- Phase B: grouped MLP ----------------
            for s in range(4):
                gram(s)

            for l in range(TWO_N):
                w0a = w0p.tile([128, H], F16, name="w0a", tag="w0a")
                nc.sync.dma_start(out=w0a, in_=W0f[l, 0:128, :])
                w0b = w0p.tile([8, H], F16, name="w0b", tag="w0b")
                nc.sync.dma_start(out=w0b, in_=W0f[l, 128:NPAIR, :])
                w1t = w12p.tile([128, 8 * H], F16, name="w1t", tag="w12")
                nc.sync.dma_start(out=w1t, in_=W1t[l, :, :])
                w2t = w12p.tile([128, 8 * H], F16, name="w2t", tag="w12")
                nc.sync.dma_start(out=w2t, in_=W2t[l, :, :])
                w3 = w3p.tile([128, 8 * TWO_N], F16, name="w3t", tag="w3")
                nc.sync.dma_start(out=w3, in_=W3t[l, :, :])

                for bt in range(N_BT):
                    bs = 512 * bt
                    # L0: scalT -> h0
                    h0 = hp.tile([128, 8, 512], F16, name="h0", tag="h")
                    for ot in range(8):
                        pt = ps.tile([128, 512], F32, name="mlp", tag="mlp",
                                     bufs=4)
                        nc.tensor.matmul(
                            pt[:, :], w0a[:, 128 * ot:128 * (ot + 1)],
                            scalT0[:, bs:bs + 512], start=True, stop=False)
                        nc.tensor.matmul(
                            pt[:, :], w0b[:, 128 * ot:128 * (ot + 1)],
                            scalT1[:, bs:bs + 512], start=False, stop=True)
                        nc.scalar.activation(
                            h0[:, ot, :], pt[:, :], TANH,
                            bias=b012_all[:, 0, ot, l:l + 1])
                    # L1, L2
                    hin = h0
                    for li, wt in ((1, w1t), (2, w2t)):
                        hout = hp.tile([128, 8, 512], F16,
                                       name=f"h{li}", tag="h")
                        for ot in range(8):
                            pt = ps.tile([128, 512], F32, name="mlp",
                                         tag="mlp", bufs=4)
                            for kt in range(8):
                                nc.tensor.matmul(
                                    pt[:, :],
                                    wt[:, kt * H + 128 * ot:
                                       kt * H + 128 * (ot + 1)],
                                    hin[:, kt, :],
                                    start=(kt == 0), stop=(kt == 7))
                            nc.scalar.activation(
                                hout[:, ot, :], pt[:, :], TANH,
                                bias=b012_all[:, li, ot, l:l + 1])
                        hin = hout
                    # L3 -> coeffs [16, 512] + bias, transpose to batch-major
                    p3 = ps.tile([16, 512], F32, name="p3", tag="p3", bufs=1)
                    for kt in range(8):
                        nc.tensor.matmul(
                            p3[:, :], w3[:, 16 * kt:16 * (kt + 1)],
                            hin[:, kt, :], start=(kt == 0), stop=(kt == 7))
                    csb = wk.tile([16, 512], F16, name="csb", tag="csb")
                    nc.scalar.add(csb[:, :], p3[:, :], b3_all[:, l:l + 1])
                    for j in range(4):
                        tp = ps.tile([128, 128], F16, name="tpg", tag="tp",
                                     bufs=2)
                        nc.tensor.transpose(
                            tp[:, 0:16], csb[:, 128 * j:128 * (j + 1)],
                            ident[0:16, 0:16])
                        sub = 4 * bt + j
                        nc.scalar.copy(
                            coeff[sub][:, 16 * l:16 * (l + 1)], tp[:, 0:16])

                    if l == 0:
                        # l=0 is Gram-bound: emit the next Gram group here
                        # and defer finals to the end of the group loop.
                        if bt < 3:
                            for s in range(4 * bt + 4, 4 * bt + 8):
                                gram(s)
                    else:
                        for s in range(4 * bt, 4 * bt + 4):
                            eng = nc.gpsimd if s % 4 == 3 else nc.vector
                            if l == TWO_N - 1:
                                eng = nc.gpsimd if s % 2 == 1 else nc.vector
                            final_unit(l, s, eng)

                if l == 0:
                    for s in range(N_SUB):
                        final_unit(l, s, nc.gpsimd if s % 4 == 3 else nc.vector)

    nc.finalize()
    return nc


_NC = None


def prepare_shared(W0, b0, W1, b1, W2, b2, W3, b3):
    """Host-side weight prep: fold W0 over symmetric pairs into the
    (dl, n) order, pre-tile W1/W2/W3 by K-tile, all fp16."""
    W0 = np.asarray(W0, np.float32).reshape(TWO_N, TWO_N, TWO_N, H)
    W0f = np.zeros((TWO_N, NPAIR, H), np.float32)
    for dl in range(TWO_N):
        for n in range(TWO_N - dl):
            j = OFF[dl] + n
            W0f[:, j, :] = W0[:, n, n + dl, :]
            if dl > 0:
                W0f[:, j, :] += W0[:, n + dl, n, :]

    def tile_k(Wm, width):
        # [2N, 1024, width] -> [2N, 128, 8*width] with [l, p, kt*width+m]
        Wm = np.asarray(Wm, np.float32).reshape(TWO_N, 8, 128, width)
        return np.ascontiguousarray(
            Wm.transpose(0, 2, 1, 3).reshape(TWO_N, 128, 8 * width)
            .astype(np.float16))

    # b012[p, li, ot, l] = b_li[l, ot*128 + p]
    b012 = np.zeros((128, 3, 8, TWO_N), np.float32)
    for li, bsrc in enumerate((b0, b1, b2)):
        bb = np.asarray(bsrc, np.float32).reshape(TWO_N, 8, 128)
        b012[:, li, :, :] = bb.transpose(2, 1, 0)
    return {
        "W0f": np.ascontiguousarray(W0f.astype(np.float16)),
        "W1t": tile_k(W1, H),
        "W2t": tile_k(W2, H),
        "W3t": tile_k(W3, TWO_N),
        "b012": np.ascontiguousarray(b012.reshape(128, 3 * 8 * TWO_N)),
        "b3T": np.ascontiguousarray(np.asarray(b3, np.float32).T),
    }


def prepare_in_maps(x, shared):
    x = np.asarray(x, np.float32)
    xdm = np.ascontiguousarray(
        x.reshape(B, TWO_N, D).transpose(0, 2, 1).reshape(B, TWO_N * D)
        .astype(np.float16))
    in_maps = []
    for c in range(N_CORES):
        m = dict(shared)
        m["xdm"] = xdm[B_LOC * c:B_LOC * (c + 1), :]
        in_maps.append(m)
    return in_maps


def kernel(x, W0, b0, W1, b1, W2, b2, W3, b3):
    global _NC
    if _NC is None:
        _NC = _build_program()
    shared = prepare_shared(W0, b0, W1, b1, W2, b2, W3, b3)
    in_maps = prepare_in_maps(x, shared)
    res = run_bass_kernel_spmd(_NC, in_maps, list(range(N_CORES)))
    return np.concatenate([res.results[c]["y"] for c in range(N_CORES)],
                          axis=0)


# revision 19
# speedup vs baseline: 1.0122x; 1.0122x over previous
"""Trainium2 Bass kernel for nn_EquivarianceNetwork (grouped 4-layer MLP).

Math (per sample b, TWO_N=16 groups, D=64):
  xr = x.reshape(B, 16, 64)
  scalars[b, n, m] = <xr[b,n], xr[b,m]>                    # 136 unique pairs
  per group l: h = tanh(...W0/W1/W2...), coeffs = h @ W3 + b3   # [B, 16]
  out[b, l*64:(l+1)*64] = sum_n coeffs[l,b,n] * xr[b,n]

Distribution: data-parallel over batch across 8 cores (weights replicated).
Per core B_local = 2048.

Design notes (v1):
  - All matmuls fp16 (1 cyc/row on PE, same rate as fp32r, 11-bit mantissa);
    weights host-pretiled to [K-part, kt, M] fp16 -> half the DMA bytes.
  - x is host-transposed to d-major per sample (xdm[b, d*16+n]), fp16. One
    resident SBUF copy serves both the Gram products and the final
    contraction; no per-stage x re-DMA.
  - Scalars: only the 136 unique pairs, ordered by diagonal offset dl so
    each Gram reduce writes a contiguous slice. W0 is host-folded and
    row-reordered to match. Gram = 16 fp16 muls (2x DVE mode, d-major
    slices) + a flat fp16 halving add-tree over d (2x mode).
  - Finals: coeffs stored batch-major fp16 [128, 256]; unit (l,s) is one
    broadcast-outer mul (2x mode, ~0.7us) + one reduce-X into f32.
  - DVE/GPSIMD/ACT all end far below PE (~2.1ms); PE stream is the
    critical path.
"""
import numpy as np

import concourse.bass as bass
import concourse.mybir as mybir
import concourse.tile as tile
from concourse import bacc
from concourse.bass_utils import run_bass_kernel_spmd
from concourse.masks import make_identity

F32 = mybir.dt.float32
F16 = mybir.dt.float16
TANH = mybir.ActivationFunctionType.Tanh

N_CORES = 8
B = 16384
TWO_N = 16
D = 64
B_LOC = B // N_CORES          # 2048
N_SUB = B_LOC // 128          # 16 subtiles of 128 samples
N_BT = B_LOC // 512           # 4 batch tiles of 512 (matmul free dim)
H = 1024                      # hidden width
NPAIR = 136                   # unique Gram pairs
# pair order: j = OFF[dl] + n  <->  scal_j = <xr[n], xr[n+dl]>
OFF = [0]
for _dl in range(1, 16):
    OFF.append(OFF[-1] + (16 - _dl + 1))


def _ap3(t, off, s1, n1, s2, n2):
    return bass.AP(tensor=t.tensor, offset=t.offset + off,
                   ap=[t.ap[0], [s1, n1], [s2, n2]])


def _build_program():
    nc = bacc.Bacc()

    xdm = nc.declare_dram_parameter("xdm", [B_LOC, TWO_N * D], F16, isOutput=False)
    xnm = nc.declare_dram_parameter("xnm", [B_LOC, TWO_N * D], F16, isOutput=False)
    W0f = nc.declare_dram_parameter("W0f", [TWO_N, NPAIR, H], F16, isOutput=False)
    W1t = nc.declare_dram_parameter("W1t", [TWO_N, 128, 8 * H], F16, isOutput=False)
    W2t = nc.declare_dram_parameter("W2t", [TWO_N, 128, 8 * H], F16, isOutput=False)
    W3t = nc.declare_dram_parameter("W3t", [TWO_N, 128, 8 * TWO_N], F16,
                                    isOutput=False)
    b012 = nc.declare_dram_parameter("b012", [128, 3 * 8 * TWO_N], F32,
                                     isOutput=False)
    b3T = nc.declare_dram_parameter("b3T", [TWO_N, TWO_N], F32, isOutput=False)
    y = nc.declare_dram_parameter("y", [B_LOC, TWO_N * D], F32, isOutput=True)

    with tile.TileContext(nc) as tc:
        with tc.tile_pool(name="res", bufs=1) as res, \
             tc.tile_pool(name="gw", bufs=1) as gw, \
             tc.tile_pool(name="wk", bufs=2) as wk, \
             tc.tile_pool(name="w0p", bufs=2) as w0p, \
             tc.tile_pool(name="w12p", bufs=4) as w12p, \
             tc.tile_pool(name="w3p", bufs=2) as w3p, \
             tc.tile_pool(name="hp", bufs=2) as hp, \
             tc.tile_pool(name="fin", bufs=4) as finp, \
             tc.tile_pool(name="ps", bufs=4, space="PSUM") as ps:

            ident = res.tile([128, 128], F16)
            make_identity(nc, ident)

            # ---- x: resident d-major fp16 (finals); n-major streamed (gram)
            xh = [res.tile([128, TWO_N * D], F16, name=f"xh{s}")
                  for s in range(N_SUB)]

            # resident: transposed scalars, fp16
            scalT0 = res.tile([128, B_LOC], F16, name="scalT0")
            scalT1 = res.tile([8, B_LOC], F16, name="scalT1")
            # resident: coeffs batch-major per subtile [128, 256] (col 16l+n)
            coeff = [res.tile([128, 256], F16, name=f"coeff{s}")
                     for s in range(N_SUB)]

            # ---------------- Gram for one subtile of 128 samples ----------
            # n-major x: prod[p, j*64+d] = xn[p, n*64+d] * xn[p, (n+dl)*64+d]
            # (j = OFF[dl]+n); flat fp16 muls (2x mode), then a halving
            # add-tree over d (inner dim) -> sbm[p, 136]
            def gram(s):
                xn = gw.tile([128, TWO_N * D], F16, name="xn", tag="xn",
                             bufs=4)
                nc.sync.dma_start(out=xn, in_=xnm[128 * s:128 * (s + 1), :])
                prod = gw.tile([128, NPAIR * 64], F16, name="prod", tag="prod")
                for dl in range(TWO_N):
                    npair = TWO_N - dl
                    meng = nc.gpsimd if dl < 5 else nc.vector
                    meng.tensor_mul(
                        prod[:, OFF[dl] * 64:(OFF[dl] + npair) * 64],
                        xn[:, 0:npair * 64],
                        xn[:, dl * 64:(dl + npair) * 64],
                    )
                sbm = gw.tile([128, NPAIR], F16, name="sbm", tag="sbm", bufs=2)
                src = prod
                d2 = 32
                for st in range(6):
                    dst = (gw.tile([128, NPAIR * d2], F16, name=f"t{st}",
                                   tag=f"t{st}")
                           if st < 5 else sbm)
                    nc.vector.tensor_add(
                        _ap3(dst, 0, d2, NPAIR, 1, d2),
                        _ap3(src, 0, 2 * d2, NPAIR, 1, d2),
                        _ap3(src, d2, 2 * d2, NPAIR, 1, d2))
                    src = dst
                    d2 //= 2
                pt = ps.tile([128, 256], F16, name="tpg", tag="tp", bufs=1)
                nc.tensor.transpose(pt[:, 0:128], sbm[:, 0:128], ident)
                nc.scalar.copy(scalT0[:, 128 * s:128 * (s + 1)], pt[:, 0:128])
                nc.tensor.transpose(pt[0:8, 128:256], sbm[:, 128:NPAIR], ident)
                nc.scalar.copy(scalT1[:, 128 * s:128 * (s + 1)],
                               pt[0:8, 128:256])

            # ---- final contraction for one (l, subtile):
            # y[bsub, l*64+d] = sum_n coeff[b, 16l+n] * xh[b, d*16+n]
            def final_unit(l, s, eng):
                c = coeff[s]
                prodf = finp.tile([128, TWO_N * D], F16, name="prodf",
                                  tag="prodf", bufs=3)
                eng.tensor_mul(
                    _ap3(prodf, 0, 16, 64, 1, 16),
                    _ap3(xh[s], 0, 16, 64, 1, 16),
                    _ap3(c, 16 * l, 0, 64, 1, 16),
                )
                fcol = finp.tile([128, D], F32, name="fcol", tag="fcol")
                nc.vector.tensor_reduce(
                    fcol[:, :], _ap3(prodf, 0, 16, 64, 1, 16),
                    axis=mybir.AxisListType.X, op=mybir.AluOpType.add)
                nc.sync.dma_start(
                    out=y[128 * s:128 * (s + 1), D * l:D * (l + 1)],
                    in_=fcol[:, :])

            # ---------------- Phase B: grouped MLP ----------------
            for s in range(4):
                gram(s)

            # d-major x + biases: needed from the finals (l>=1) onward
            for s in range(N_SUB):
                nc.sync.dma_start(out=xh[s], in_=xdm[128 * s:128 * (s + 1), :])
            b012_all = res.tile([128, 3, 8, TWO_N], F32)
            nc.sync.dma_start(
                out=b012_all,
                in_=b012[:, :].rearrange("p (li ot l) -> p li ot l",
                                         li=3, ot=8))
            b3_all = res.tile([16, TWO_N], F32)   # [n, l]
            nc.sync.dma_start(out=b3_all, in_=b3T[:, :])

            for l in range(TWO_N):
                w0a = w0p.tile([128, H], F16, name="w0a", tag="w0a")
                nc.sync.dma_start(out=w0a, in_=W0f[l, 0:128, :])
                w0b = w0p.tile([8, H], F16, name="w0b", tag="w0b")
                nc.sync.dma_start(out=w0b, in_=W0f[l, 128:NPAIR, :])
                w1t = w12p.tile([128, 8 * H], F16, name="w1t", tag="w12")
                nc.sync.dma_start(out=w1t, in_=W1t[l, :, :])
                w2t = w12p.tile([128, 8 * H], F16, name="w2t", tag="w12")
                nc.sync.dma_start(out=w2t, in_=W2t[l, :, :])
                w3 = w3p.tile([128, 8 * TWO_N], F16, name="w3t", tag="w3")
                nc.sync.dma_start(out=w3, in_=W3t[l, :, :])

                for bt in range(N_BT):
                    bs = 512 * bt
                    # L0: scalT -> h0
                    h0 = hp.tile([128, 8, 512], F16, name="h0", tag="h")
                    for ot in range(8):
                        pt = ps.tile([128, 512], F32, name="mlp", tag="mlp",
                                     bufs=5)
                        nc.tensor.matmul(
                            pt[:, :], w0a[:, 128 * ot:128 * (ot + 1)],
                            scalT0[:, bs:bs + 512], start=True, stop=False)
                        nc.tensor.matmul(
                            pt[:, :], w0b[:, 128 * ot:128 * (ot + 1)],
                            scalT1[:, bs:bs + 512], start=False, stop=True)
                        nc.scalar.activation(
                            h0[:, ot, :], pt[:, :], TANH,
                            bias=b012_all[:, 0, ot, l:l + 1])
                    # L1, L2
                    hin = h0
                    for li, wt in ((1, w1t), (2, w2t)):
                        hout = hp.tile([128, 8, 512], F16,
                                       name=f"h{li}", tag="h")
                        for ot in range(8):
                            pt = ps.tile([128, 512], F32, name="mlp",
                                         tag="mlp", bufs=5)
                            for kt in range(8):
                                nc.tensor.matmul(
                                    pt[:, :],
                                    wt[:, kt * H + 128 * ot:
                                       kt * H + 128 * (ot + 1)],
                                    hin[:, kt, :],
                                    start=(kt == 0), stop=(kt == 7))
                            nc.scalar.activation(
                                hout[:, ot, :], pt[:, :], TANH,
                                bias=b012_all[:, li, ot, l:l + 1])
                        hin = hout
                    # L3 -> coeffs [16, 512] + bias, transpose to batch-major
                    p3 = ps.tile([16, 512], F32, name="p3", tag="p3", bufs=1)
                    for kt in range(8):
                        nc.tensor.matmul(
                            p3[:, :], w3[:, 16 * kt:16 * (kt + 1)],
                            hin[:, kt, :], start=(kt == 0), stop=(kt == 7))
                    csb = wk.tile([16, 512], F16, name="csb", tag="csb")
                    nc.scalar.add(csb[:, :], p3[:, :], b3_all[:, l:l + 1])
                    for j in range(4):
                        tp = ps.tile([128, 256], F16, name="tpg", tag="tp",
                                     bufs=1)
                        nc.tensor.transpose(
                            tp[:, 0:16], csb[:, 128 * j:128 * (j + 1)],
                            ident[0:16, 0:16])
                        sub = 4 * bt + j
                        nc.scalar.copy(
                            coeff[sub][:, 16 * l:16 * (l + 1)], tp[:, 0:16])

                    if l == 0:
                        # l=0 is Gram-bound: emit the next Gram group here
                        # and defer finals to the end of the group loop.
                        if bt < 3:
                            for s in range(4 * bt + 4, 4 * bt + 8):
                                gram(s)
                    else:
                        for s in range(4 * bt, 4 * bt + 4):
                            eng = nc.gpsimd if s % 8 == 7 else nc.vector
                            if l == TWO_N - 1:
                                eng = nc.gpsimd if s % 2 == 1 else nc.vector
                            final_unit(l, s, eng)

                if l == 0:
                    for s in range(N_SUB):
                        final_unit(l, s, nc.gpsimd if s % 4 == 3 else nc.vector)

    nc.finalize()
    return nc


_NC = None


def prepare_shared(W0, b0, W1, b1, W2, b2, W3, b3):
    """Host-side weight prep: fold W0 over symmetric pairs into the
    (dl, n) order, pre-tile W1/W2/W3 by K-tile, all fp16."""
    W0 = np.asarray(W0, np.float32).reshape(TWO_N, TWO_N, TWO_N, H)
    W0f = np.zeros((TWO_N, NPAIR, H), np.float32)
    for dl in range(TWO_N):
        for n in range(TWO_N - dl):
            j = OFF[dl] + n
            W0f[:, j, :] = W0[:, n, n + dl, :]
            if dl > 0:
                W0f[:, j, :] += W0[:, n + dl, n, :]

    def tile_k(Wm, width):
        # [2N, 1024, width] -> [2N, 128, 8*width] with [l, p, kt*width+m]
        Wm = np.asarray(Wm, np.float32).reshape(TWO_N, 8, 128, width)
        return np.ascontiguousarray(
            Wm.transpose(0, 2, 1, 3).reshape(TWO_N, 128, 8 * width)
            .astype(np.float16))

    # b012[p, li, ot, l] = b_li[l, ot*128 + p]
    b012 = np.zeros((128, 3, 8, TWO_N), np.float32)
    for li, bsrc in enumerate((b0, b1, b2)):
        bb = np.asarray(bsrc, np.float32).reshape(TWO_N, 8, 128)
        b012[:, li, :, :] = bb.transpose(2, 1, 0)
    return {
        "W0f": np.ascontiguousarray(W0f.astype(np.float16)),
        "W1t": tile_k(W1, H),
        "W2t": tile_k(W2, H),
        "W3t": tile_k(W3, TWO_N),
        "b012": np.ascontiguousarray(b012.reshape(128, 3 * 8 * TWO_N)),
        "b3T": np.ascontiguousarray(np.asarray(b3, np.float32).T),
    }


def prepare_in_maps(x, shared):
    x = np.asarray(x, np.float32)
    xnm = np.ascontiguousarray(x.astype(np.float16))
    xdm = np.ascontiguousarray(
        x.reshape(B, TWO_N, D).transpose(0, 2, 1).reshape(B, TWO_N * D)
        .astype(np.float16))
    in_maps = []
    for c in range(N_CORES):
        m = dict(shared)
        m["xdm"] = xdm[B_LOC * c:B_LOC * (c + 1), :]
        m["xnm"] = xnm[B_LOC * c:B_LOC * (c + 1), :]
        in_maps.append(m)
    return in_maps


def kernel(x, W0, b0, W1, b1, W2, b2, W3, b3):
    global _NC
    if _NC is None:
        _NC = _build_program()
    shared = prepare_shared(W0, b0, W1, b1, W2, b2, W3, b3)
    in_maps = prepare_in_maps(x, shared)
    res = run_bass_kernel_spmd(_NC, in_maps, list(range(N_CORES)))
    return np.concatenate([res.results[c]["y"] for c in range(N_CORES)],
                          axis=0)
